# revision 6
# baseline (speedup 1.0000x reference)
"""GATv2 (2-layer, 4 heads) + linear classifier on Trainium2, 8-core SPMD.

Sharding: nodes are partitioned contiguously across 8 cores (2500 nodes/core).
Edges are routed to the core that owns their destination node, so the
segment-softmax and scatter-add stay core-local.  The only cross-core
exchange is one AllGather per GAT layer of the (att-scaled) source-side
linear transform table, which every core then gathers rows from by edge
source id (the "halo exchange" for cut edges).

Math decomposition (per layer, per head h with C=128 channels), using
leaky_relu(z) = 0.6 z + 0.4 |z| for slope 0.2:
  score_e = att_h . leaky_relu(z) = 0.6 * att.z + 0.4 * att.|z|
with z = xl[src] + xr[dst].  The linear part att.z = adl[src] + adr[dst]
where adl/adr are per-head att-weighted row sums of xl/xr, stored as 4
extra table columns so one gather brings both the 512 features and the
sums.  The |z| part costs one Abs (scalar engine) + one multiply by the
replicated att row + one per-head reduce (vector engine) per edge tile.
Softmax max-subtraction is skipped (scores are in [-6, 6], exact in fp32).
Per 128-dst block, a one-hot (edge -> local segment) matrix turns the
segment sum of exp weights and of exp-weighted features into PE matmuls
accumulated in PSUM; the block epilogue divides by the denominator, adds
the output bias, applies ELU, and stores the block transposed
(feature-major) for the next layer's matmuls.
"""

import numpy as np

N_NODES = 20000
IN_CH = 55
HID = 128
HEADS = 4
D = HID * HEADS  # 512
OUT_CH = 49
NCORES = 8
PER_CORE = N_NODES // NCORES  # 2500
BLOCK = 128
NEG_SLOPE = 0.2
W_TAB = D + HEADS  # 516: features + per-head row sums


# ---------------------------------------------------------------- host prep


def _att_fold(w, b, att_flat):
    return (np.asarray(w, np.float32) * att_flat[None, :]).astype(np.float32), (
        np.asarray(b, np.float32) * att_flat
    ).astype(np.float32)


def _plan_edges(src, dst, n_nodes, per_core, n_cores):
    """Route edges to (core, block) by dst; pad each block to a multiple of
    128 edge slots, uniformly across cores (SPMD program must be identical).

    Returns dict with per-core [128, TT] arrays (esrc, edst, logvalid) and
    the static per-block tile counts.
    """
    blocks_per_core = (per_core + BLOCK - 1) // BLOCK
    core = dst // per_core
    dst_local = dst - core * per_core
    blk = dst_local // BLOCK

    # counts[c][b]
    counts = np.zeros((n_cores, blocks_per_core), dtype=np.int64)
    np.add.at(counts, (core, blk), 1)
    tiles = np.maximum(1, -(-counts.max(axis=0) // 128))  # per-block tile count
    offs = np.concatenate([[0], np.cumsum(tiles)])[:-1]
    tt = int(tiles.sum())

    esrc = np.zeros((n_cores, 128, tt), dtype=np.int32)
    edst = np.zeros((n_cores, 128, tt), dtype=np.int32)
    lv = np.full((n_cores, 128, tt), -1e30, dtype=np.float32)

    order = np.lexsort((blk, core))
    src_s, core_s, blk_s, dstl_s = src[order], core[order], blk[order], dst_local[order]
    # boundaries per (core, blk)
    key = core_s * blocks_per_core + blk_s
    bounds = np.searchsorted(key, np.arange(n_cores * blocks_per_core + 1))
    for c in range(n_cores):
        for b in range(blocks_per_core):
            k = c * blocks_per_core + b
            lo, hi = bounds[k], bounds[k + 1]
            cnt = hi - lo
            nslots = int(tiles[b]) * 128
            s = np.zeros(nslots, dtype=np.int32)
            d_ = np.zeros(nslots, dtype=np.int32)
            v = np.full(nslots, -1e30, dtype=np.float32)
            s[:cnt] = src_s[lo:hi]
            d_[:cnt] = dstl_s[lo:hi]
            # dummy slots: dst_local must stay inside this block so the
            # one-hot lands in a valid segment (weight is 0 anyway)
            d_[cnt:] = b * BLOCK if b * BLOCK < per_core else 0
            v[:cnt] = 0.0
            o = int(offs[b])
            t = int(tiles[b])
            esrc[c][:, o : o + t] = s.reshape(t, 128).T
            edst[c][:, o : o + t] = d_.reshape(t, 128).T
            lv[c][:, o : o + t] = v.reshape(t, 128).T
    return dict(
        tiles=[int(t) for t in tiles],
        offs=[int(o) for o in offs],
        tt=tt,
        blocks=blocks_per_core,
        esrc=esrc,
        edst=edst,
        logvalid=lv,
    )


def preprocess(x, edge_index, w1_l, b1_l, w1_r, b1_r, att1, bias1,
               w2_l, b2_l, w2_r, b2_r, att2, bias2, w_cls, b_cls,
               n_cores=NCORES):
    x = np.asarray(x, np.float32)
    n = x.shape[0]
    per = n // n_cores
    ei = np.asarray(edge_index).astype(np.int64)
    loops = np.arange(n, dtype=np.int64)
    src = np.concatenate([ei[0], loops])
    dst = np.concatenate([ei[1], loops])

    att1_flat = np.asarray(att1, np.float32).reshape(-1)
    att2_flat = np.asarray(att2, np.float32).reshape(-1)

    w1l = np.asarray(w1_l, np.float32); b1l = np.asarray(b1_l, np.float32)
    w1r = np.asarray(w1_r, np.float32); b1r = np.asarray(b1_r, np.float32)
    w2l = np.asarray(w2_l, np.float32); b2l = np.asarray(b2_l, np.float32)
    w2r = np.asarray(w2_r, np.float32); b2r = np.asarray(b2_r, np.float32)

    plan = _plan_edges(src, dst, n, per, n_cores)

    in_ch = x.shape[1]
    w1l_aug = np.concatenate([w1l, b1l[None, :]], axis=0)  # [in+1, 512]
    w1r_aug = np.concatenate([w1r, b1r[None, :]], axis=0)

    rep = lambda v: np.broadcast_to(np.asarray(v, np.float32)[None, :], (128, v.shape[0])).copy()

    shared = {
        "w1l_aug": w1l_aug,
        "w1r_aug": w1r_aug,
        "w2l": np.asarray(w2l, np.float32),
        "w2r": np.asarray(w2r, np.float32),
        "b2l_row": b2l[None, :].astype(np.float32),
        "b2r_row": b2r[None, :].astype(np.float32),
        "wcls": np.asarray(w_cls, np.float32),
        "bcls_row": np.asarray(b_cls, np.float32)[None, :],
        "att1_rep": rep(att1_flat),
        "att2_rep": rep(att2_flat),
        "bias1_rep": rep(np.asarray(bias1, np.float32)),
        "bias2_rep": rep(np.asarray(bias2, np.float32)),
    }
    in_maps = []
    for c in range(n_cores):
        xa = np.concatenate(
            [x[c * per : (c + 1) * per].T, np.ones((1, per), np.float32)], axis=0
        )  # [in+1, per]
        m = dict(shared)
        m["x_aug"] = np.ascontiguousarray(xa)
        m["esrc"] = plan["esrc"][c]
        m["edst"] = plan["edst"][c]
        m["logvalid"] = plan["logvalid"][c]
        in_maps.append(m)
    meta = dict(
        n=n, per=per, in_ch=in_ch, tiles=plan["tiles"], offs=plan["offs"],
        tt=plan["tt"], blocks=plan["blocks"], n_cores=n_cores,
    )
    return in_maps, meta


# ---------------------------------------------------------------- device


def build_program(meta, bench=False):
    import contextlib
    import concourse.bass as bass
    import concourse.tile as tile
    import concourse.mybir as mybir
    from concourse import bacc
    from concourse.masks import make_identity

    f32 = mybir.dt.float32
    i32 = mybir.dt.int32

    n = meta["n"]
    per = meta["per"]
    in_ch = meta["in_ch"]
    tiles = meta["tiles"]
    offs = meta["offs"]
    tt = meta["tt"]
    blocks = meta["blocks"]
    n_cores = meta["n_cores"]

    nt_full, nt_rem = divmod(per, 128)
    node_tiles = [(i * 128, 128) for i in range(nt_full)]
    if nt_rem:
        node_tiles.append((nt_full * 128, nt_rem))

    nc = bacc.Bacc("TRN2", target_bir_lowering=False, debug=False, num_devices=n_cores)

    # register a -1.0 const AP so scalar.add(x, -1.0) lowers on the ACT engine
    _cm1 = nc.alloc_sbuf_tensor("const-float32-neg1", [128, 1], f32)
    nc.gpsimd.memset(_cm1.ap(), -1.0)
    nc.const_aps.aps[(f32, -1.0)] = _cm1.ap()

    def din(name, shape, dt=f32):
        return nc.dram_tensor(name, shape, dt, kind="ExternalInput").ap()

    x_aug = din("x_aug", [in_ch + 1, per])
    w1l_aug = din("w1l_aug", [in_ch + 1, D])
    w1r_aug = din("w1r_aug", [in_ch + 1, D])
    w2l = din("w2l", [D, D])
    w2r = din("w2r", [D, D])
    b2l_row = din("b2l_row", [1, D])
    b2r_row = din("b2r_row", [1, D])
    wcls = din("wcls", [D, OUT_CH])
    bcls_row = din("bcls_row", [1, OUT_CH])
    att1_rep = din("att1_rep", [128, D])
    att2_rep = din("att2_rep", [128, D])
    bias1_rep = din("bias1_rep", [128, D])
    bias2_rep = din("bias2_rep", [128, D])
    esrc = din("esrc", [128, tt], i32)
    edst = din("edst", [128, tt], i32)
    logvalid = din("logvalid", [128, tt])
    kreps = din("kreps", [1, 8], i32) if bench else None

    out = nc.dram_tensor("out", [per, OUT_CH], f32, kind="ExternalOutput").ap()

    with tile.TileContext(nc) as tc:
        with (
            tc.tile_pool(name="dram", bufs=1, space="DRAM") as dram,
            tc.tile_pool(name="consts", bufs=1) as consts,
            tc.tile_pool(name="w2", bufs=1) as w2pool,
        ):
            # ---------- persistent DRAM intermediates
            xl1_loc = dram.tile([per, W_TAB], f32)
            xr1_loc = dram.tile([per, W_TAB], f32)
            xl1_full = dram.tile([n, W_TAB], f32, addr_space="Shared")
            h1T = dram.tile([D, blocks * BLOCK], f32)
            xl2_loc = dram.tile([per, W_TAB], f32)
            xr2_loc = dram.tile([per, W_TAB], f32)
            xl2_full = dram.tile([n, W_TAB], f32, addr_space="Shared")
            h2T = dram.tile([D, blocks * BLOCK], f32)

            # ---------- constants in SBUF
            identity = consts.tile([128, 128], f32)
            make_identity(nc, identity[:])
            iota_f = consts.tile([128, 128], f32)
            iota_i = consts.tile([128, 128], i32)
            nc.gpsimd.iota(iota_i[:], pattern=[[1, 128]], base=0, channel_multiplier=0)
            nc.vector.tensor_copy(iota_f[:], iota_i[:])
            ones_sb = consts.tile([1, 128], f32)
            nc.vector.memset(ones_sb[:], 1.0)

            if bench:
                kt = consts.tile([1, 8], i32)
                nc.sync.dma_start(kt[:], kreps[:])
                kregs = [nc.values_load(kt[0:1, j : j + 1]) for j in range(5)]

            def seg_loop(j):
                if bench:
                    return tc.For_i(0, kregs[j], 1)
                return contextlib.nullcontext()

            w1l_sb = consts.tile([in_ch + 1, D], f32)
            nc.sync.dma_start(w1l_sb[:], w1l_aug[:])
            w1r_sb = consts.tile([in_ch + 1, D], f32)
            nc.sync.dma_start(w1r_sb[:], w1r_aug[:])
            att1_sb = consts.tile([128, D], f32)
            nc.sync.dma_start(att1_sb[:], att1_rep[:])
            att2_sb = consts.tile([128, D], f32)
            nc.sync.dma_start(att2_sb[:], att2_rep[:])
            bias1_sb = consts.tile([128, D], f32)
            nc.sync.dma_start(bias1_sb[:], bias1_rep[:])
            bias2_sb = consts.tile([128, D], f32)
            nc.sync.dma_start(bias2_sb[:], bias2_rep[:])

            # w2 weights: [512, 512] split into 4 chunks of [128, 512]
            w2l_ch = []
            w2r_ch = []
            wcls_ch = []
            for k in range(4):
                t1 = w2pool.tile([128, D], f32, name=f"w2l_{k}")
                nc.sync.dma_start(t1[:], w2l[k * 128 : (k + 1) * 128, :])
                w2l_ch.append(t1)
                t2 = w2pool.tile([128, D], f32, name=f"w2r_{k}")
                nc.sync.dma_start(t2[:], w2r[k * 128 : (k + 1) * 128, :])
                w2r_ch.append(t2)
                t3 = w2pool.tile([128, OUT_CH], f32, name=f"wcls_{k}")
                nc.sync.dma_start(t3[:], wcls[k * 128 : (k + 1) * 128, :])
                wcls_ch.append(t3)
            b2l_sb = w2pool.tile([1, D], f32)
            nc.sync.dma_start(b2l_sb[:], b2l_row[:])
            b2r_sb = w2pool.tile([1, D], f32)
            nc.sync.dma_start(b2r_sb[:], b2r_row[:])
            bcls_sb = w2pool.tile([1, OUT_CH], f32)
            nc.sync.dma_start(bcls_sb[:], bcls_row[:])

            # ================= stage 0: layer-1 dense transforms (local rows)
            with (
                tc.tile_pool(name="s0_in", bufs=3) as s0in,
                tc.tile_pool(name="s0_ps", bufs=2, space="PSUM") as s0ps,
                tc.tile_pool(name="s0_out", bufs=3) as s0out,
                seg_loop(0),
            ):
                for base, m in node_tiles:
                    lx = s0in.tile([in_ch + 1, 128], f32, tag="lx")
                    nc.sync.dma_start(lx[:, :m], x_aug[:, base : base + m])
                    for wsb, table in ((w1l_sb, xl1_loc), (w1r_sb, xr1_loc)):
                        ps = s0ps.tile([128, D], f32, space="PSUM", tag="ps")
                        nc.tensor.matmul(
                            ps[:m, :], lhsT=lx[:, :m], rhs=wsb[:], start=True, stop=True
                        )
                        sb = s0out.tile([128, W_TAB], f32, tag="sb")
                        nc.scalar.copy(sb[:m, :D], ps[:m, :])
                        tmp = s0out.tile([128, D], f32, tag="tmp")
                        nc.vector.tensor_mul(tmp[:m, :], sb[:m, :D], att1_sb[:m, :])
                        nc.vector.reduce_sum(
                            out=sb[:m, D:W_TAB],
                            in_=tmp[:m, :].rearrange("p (h c) -> p h c", h=HEADS),
                            axis=mybir.AxisListType.X,
                        )
                        nc.sync.dma_start(table[base : base + m, :], sb[:m, :])

            # AllGather layer-1 source table
            nc.gpsimd.collective_compute(
                "AllGather",
                mybir.AluOpType.bypass,
                replica_groups=[list(range(n_cores))],
                ins=[xl1_loc.opt()],
                outs=[xl1_full.opt()],
            )

            # ================= edge phase (shared for both layers)
            def edge_phase(xl_full_ap, xr_loc_ap, att_sb, bias_sb, hT_ap, segj):
                with (
                    tc.tile_pool(name="eidx", bufs=2) as eidx,
                    tc.tile_pool(name="eg", bufs=6) as eg,
                    tc.tile_pool(name="esm", bufs=8) as esm,
                    tc.tile_pool(name="eoh", bufs=6) as eoh,
                    tc.tile_pool(name="ew", bufs=4) as ew,
                    tc.tile_pool(name="eps", bufs=2, space="PSUM") as eps,
                    tc.tile_pool(name="etail", bufs=2) as etail,
                    tc.tile_pool(name="etps", bufs=2, space="PSUM") as etps,
                    seg_loop(segj),
                ):
                    for b in range(blocks):
                        tb = tiles[b]
                        off = offs[b]
                        cbase = b * BLOCK
                        cols = min(BLOCK, per - cbase)
                        src_sb = eidx.tile([128, tb], i32, tag="src")
                        nc.sync.dma_start(src_sb[:], esrc[:, off : off + tb])
                        dst_sb = eidx.tile([128, tb], i32, tag="dst")
                        nc.sync.dma_start(dst_sb[:], edst[:, off : off + tb])
                        lv_sb = eidx.tile([128, tb], f32, tag="lv")
                        nc.sync.dma_start(lv_sb[:], logvalid[:, off : off + tb])
                        seg_f = eidx.tile([128, tb], f32, tag="seg")
                        nc.vector.tensor_copy(seg_f[:], dst_sb[:])
                        nc.vector.tensor_scalar_add(seg_f[:], seg_f[:], float(-cbase))

                        den_ps = eps.tile([128, 8], f32, space="PSUM", tag="den")
                        out_ps = eps.tile([128, D], f32, space="PSUM", tag="out")

                        for t in range(tb):
                            xg = eg.tile([128, W_TAB], f32, tag="xg")
                            nc.gpsimd.indirect_dma_start(
                                out=xg[:],
                                out_offset=None,
                                in_=xl_full_ap,
                                in_offset=bass.IndirectOffsetOnAxis(
                                    ap=src_sb[:, t : t + 1], axis=0
                                ),
                            )
                            rg = eg.tile([128, W_TAB], f32, tag="rg")
                            nc.gpsimd.indirect_dma_start(
                                out=rg[:],
                                out_offset=None,
                                in_=xr_loc_ap,
                                in_offset=bass.IndirectOffsetOnAxis(
                                    ap=dst_sb[:, t : t + 1], axis=0
                                ),
                            )
                            nc.vector.tensor_add(rg[:], rg[:], xg[:])
                            ab = ew.tile([128, D], f32, tag="ab")
                            nc.scalar.activation(
                                ab[:], rg[:, :D], mybir.ActivationFunctionType.Abs
                            )
                            nc.vector.tensor_mul(ab[:], ab[:], att_sb[:])
                            red = esm.tile([128, 4], f32, tag="red")
                            nc.vector.reduce_sum(
                                out=red[:],
                                in_=ab[:].rearrange("p (h c) -> p h c", h=HEADS),
                                axis=mybir.AxisListType.X,
                            )
                            pre = esm.tile([128, 4], f32, tag="pre")
                            nc.vector.tensor_scalar_mul(pre[:], rg[:, D:W_TAB], 1.5)
                            nc.vector.tensor_add(pre[:], pre[:], red[:])
                            exps = esm.tile([128, 4], f32, tag="exps")
                            nc.scalar.activation(
                                exps[:],
                                pre[:],
                                mybir.ActivationFunctionType.Exp,
                                bias=lv_sb[:, t : t + 1],
                                scale=0.4,
                            )
                            oh = eoh.tile([128, 128], f32, tag="oh")
                            nc.vector.tensor_tensor(
                                out=oh[:],
                                in0=seg_f[:, t : t + 1].to_broadcast([128, 128]),
                                in1=iota_f[:],
                                op=mybir.AluOpType.is_equal,
                            )
                            nc.tensor.matmul(
                                den_ps[:, :4],
                                lhsT=oh[:],
                                rhs=exps[:],
                                start=(t == 0),
                                stop=(t == tb - 1),
                            )
                            w = ew.tile([128, D], f32, tag="w")
                            nc.vector.tensor_tensor(
                                out=w[:].rearrange("p (h c) -> p h c", h=HEADS),
                                in0=xg[:, :D].rearrange("p (h c) -> p h c", h=HEADS),
                                in1=exps[:, :, None].to_broadcast([128, HEADS, HID]),
                                op=mybir.AluOpType.mult,
                            )
                            nc.tensor.matmul(
                                out_ps[:],
                                lhsT=oh[:],
                                rhs=w[:],
                                start=(t == 0),
                                stop=(t == tb - 1),
                            )

                        # ---- block epilogue
                        den_sb = esm.tile([128, 4], f32, tag="den_sb")
                        nc.vector.tensor_copy(den_sb[:], den_ps[:, :4])
                        recip = esm.tile([128, 4], f32, tag="recip")
                        nc.vector.reciprocal(recip[:], den_sb[:])
                        h = etail.tile([128, D], f32, tag="h")
                        nc.vector.tensor_tensor(
                            out=h[:].rearrange("p (h c) -> p h c", h=HEADS),
                            in0=out_ps[:].rearrange("p (h c) -> p h c", h=HEADS),
                            in1=recip[:, :, None].to_broadcast([128, HEADS, HID]),
                            op=mybir.AluOpType.mult,
                        )
                        nc.vector.tensor_add(h[:], h[:], bias_sb[:])
                        # ELU: relu(x) + exp(min(x,0)) - 1
                        neg = etail.tile([128, D], f32, tag="neg")
                        nc.vector.tensor_scalar_min(neg[:], h[:], 0.0)
                        expn = etail.tile([128, D], f32, tag="expn")
                        nc.scalar.activation(
                            expn[:], neg[:], mybir.ActivationFunctionType.Exp
                        )
                        pos = etail.tile([128, D], f32, tag="pos")
                        nc.scalar.activation(
                            pos[:], h[:], mybir.ActivationFunctionType.Relu
                        )
                        hf = etail.tile([128, D], f32, tag="hf")
                        nc.vector.tensor_add(hf[:], pos[:], expn[:])
                        nc.scalar.add(hf[:], hf[:], -1.0)
                        # transpose [128, 512] -> 4 x [128, 128] into hT
                        for q in range(4):
                            tp = etps.tile([128, 128], f32, space="PSUM", tag="tp")
                            nc.tensor.transpose(
                                tp[:], hf[:, q * 128 : (q + 1) * 128], identity[:]
                            )
                            tsb = etail.tile([128, 128], f32, tag="tsb")
                            nc.scalar.copy(tsb[:], tp[:])
                            nc.sync.dma_start(
                                hT_ap[q * 128 : (q + 1) * 128, cbase : cbase + cols],
                                tsb[:, :cols],
                            )

            edge_phase(xl1_full.opt(), xr1_loc.opt(), att1_sb, bias1_sb, h1T.opt(), 1)

            # ================= stage 2: layer-2 dense transforms from h1T
            with (
                tc.tile_pool(name="s2_in", bufs=4) as s2in,
                tc.tile_pool(name="s2_ps", bufs=2, space="PSUM") as s2ps,
                tc.tile_pool(name="s2_out", bufs=3) as s2out,
                seg_loop(2),
            ):
                for base, m in node_tiles:
                    hts = []
                    for k in range(4):
                        ht = s2in.tile([128, 128], f32, tag=f"ht{k}")
                        nc.sync.dma_start(
                            ht[:, :m], h1T[k * 128 : (k + 1) * 128, base : base + m]
                        )
                        hts.append(ht)
                    for wch, brow, table in (
                        (w2l_ch, b2l_sb, xl2_loc),
                        (w2r_ch, b2r_sb, xr2_loc),
                    ):
                        ps = s2ps.tile([128, D], f32, space="PSUM", tag="ps")
                        for k in range(4):
                            nc.tensor.matmul(
                                ps[:m, :],
                                lhsT=hts[k][:, :m],
                                rhs=wch[k][:],
                                start=(k == 0),
                                stop=False,
                            )
                        nc.tensor.matmul(
                            ps[:m, :],
                            lhsT=ones_sb[:, :m],
                            rhs=brow[:],
                            start=False,
                            stop=True,
                        )
                        sb = s2out.tile([128, W_TAB], f32, tag="sb")
                        nc.scalar.copy(sb[:m, :D], ps[:m, :])
                        tmp = s2out.tile([128, D], f32, tag="tmp")
                        nc.vector.tensor_mul(tmp[:m, :], sb[:m, :D], att2_sb[:m, :])
                        nc.vector.reduce_sum(
                            out=sb[:m, D:W_TAB],
                            in_=tmp[:m, :].rearrange("p (h c) -> p h c", h=HEADS),
                            axis=mybir.AxisListType.X,
                        )
                        nc.sync.dma_start(table[base : base + m, :], sb[:m, :])

            nc.gpsimd.collective_compute(
                "AllGather",
                mybir.AluOpType.bypass,
                replica_groups=[list(range(n_cores))],
                ins=[xl2_loc.opt()],
                outs=[xl2_full.opt()],
            )

            edge_phase(xl2_full.opt(), xr2_loc.opt(), att2_sb, bias2_sb, h2T.opt(), 3)

            # ================= classifier
            with (
                tc.tile_pool(name="c_in", bufs=4) as cin,
                tc.tile_pool(name="c_ps", bufs=2, space="PSUM") as cps,
                tc.tile_pool(name="c_out", bufs=3) as cout,
                seg_loop(4),
            ):
                for base, m in node_tiles:
                    hts = []
                    for k in range(4):
                        ht = cin.tile([128, 128], f32, tag=f"cht{k}")
                        nc.sync.dma_start(
                            ht[:, :m], h2T[k * 128 : (k + 1) * 128, base : base + m]
                        )
                        hts.append(ht)
                    ps = cps.tile([128, OUT_CH], f32, space="PSUM", tag="ps")
                    for k in range(4):
                        nc.tensor.matmul(
                            ps[:m, :],
                            lhsT=hts[k][:, :m],
                            rhs=wcls_ch[k][:],
                            start=(k == 0),
                            stop=False,
                        )
                    nc.tensor.matmul(
                        ps[:m, :],
                        lhsT=ones_sb[:, :m],
                        rhs=bcls_sb[:],
                        start=False,
                        stop=True,
                    )
                    sb = cout.tile([128, OUT_CH], f32, tag="sb")
                    nc.scalar.copy(sb[:m, :], ps[:m, :])
                    nc.sync.dma_start(out[base : base + m, :], sb[:m, :])

    nc.compile()
    return nc


# ---------------------------------------------------------------- entry

_CACHE = {}


def kernel(**inputs):
    from concourse.bass_utils import run_bass_kernel_spmd

    in_maps, meta = preprocess(**inputs)
    key = (meta["tt"], tuple(meta["tiles"]))
    if key not in _CACHE:
        _CACHE[key] = build_program(meta)
    nc = _CACHE[key]
    res = run_bass_kernel_spmd(nc, in_maps, list(range(meta["n_cores"])))
    outs = [res.results[c]["out"] for c in range(meta["n_cores"])]
    return np.concatenate(outs, axis=0)


# ---------------------------------------------------------------- numpy model
# (host-side mirror of the device math, for validation in test.py)


def numpy_model(x, edge_index, w1_l, b1_l, w1_r, b1_r, att1, bias1,
                w2_l, b2_l, w2_r, b2_r, att2, bias2, w_cls, b_cls):
    x = np.asarray(x, np.float32)
    n = x.shape[0]
    ei = np.asarray(edge_index).astype(np.int64)
    loops = np.arange(n, dtype=np.int64)
    src = np.concatenate([ei[0], loops])
    dst = np.concatenate([ei[1], loops])

    def layer(h, wl, bl, wr, br, att, bias):
        att_flat = np.asarray(att, np.float32).reshape(-1)
        xl = (h @ np.asarray(wl, np.float32) + np.asarray(bl, np.float32)).astype(np.float32)
        xr = (h @ np.asarray(wr, np.float32) + np.asarray(br, np.float32)).astype(np.float32)
        adl = (xl * att_flat).reshape(n, HEADS, HID).sum(axis=2)
        adr = (xr * att_flat).reshape(n, HEADS, HID).sum(axis=2)
        z = xl[src] + xr[dst]
        abssum = (np.abs(z) * att_flat).reshape(-1, HEADS, HID).sum(axis=2)
        adsum = adl[src] + adr[dst]
        score = 0.4 * (abssum + 1.5 * adsum)
        ex = np.exp(score)
        den = np.zeros((n, HEADS), np.float32)
        np.add.at(den, dst, ex)
        unnorm = np.zeros((n, HEADS, HID), np.float32)
        np.add.at(unnorm, dst, xl[src].reshape(-1, HEADS, HID) * ex[:, :, None])
        out = unnorm / den[:, :, None]
        out = out.reshape(n, D) + np.asarray(bias, np.float32)
        return np.where(out > 0, out, np.exp(np.minimum(out, 0)) - 1).astype(np.float32)

    h = layer(x, w1_l, b1_l, w1_r, b1_r, att1, bias1)
    h = layer(h, w2_l, b2_l, w2_r, b2_r, att2, bias2)
    return (h @ np.asarray(w_cls, np.float32) + np.asarray(b_cls, np.float32)).astype(
        np.float32
    )
